# revision 1
# baseline (speedup 1.0000x reference)
"""Trainium2 Bass kernel for nn_AttnBlock (GroupNorm + single-head 4096-token
attention + residual), sharded over 8 NeuronCores.

Sharding: data-parallel over batch B=4, sequence-parallel x2 over the 4096
query tokens -> 8 shards. Each core computes k/v for its full batch
(duplicated across the 2 token-halves) and q/attention/out-proj for its 2048
query tokens. The token axis is rolled on the host for the second half so a
single SPMD NEFF serves all cores (softmax over keys is order-invariant,
groupnorm stats are token-permutation-invariant).

Self-contained: hardcodes all shapes; only needs the concourse runtime.
"""

import numpy as np
import ml_dtypes

import concourse.bass as bass
import concourse.bacc as bacc
import concourse.tile as tile
from concourse import mybir
from concourse.bass_utils import run_bass_kernel_spmd

P = 128                 # partitions
C = 512                 # channels
N = 4096                # tokens (64*64)
NQ = 2048               # query tokens per core
CT = C // P             # 4 channel tiles
JT = N // P             # 32 key-token tiles of 128
NSTRIP = NQ // 512      # 4 query strips of 512
ITS = 512 // P          # 4 i-subtiles per strip
GS = 16                 # channels per group
NG = P // GS            # 8 groups per channel tile
EPS = 1e-6
SCALE = float(C) ** -0.5
F32 = mybir.dt.float32
BF16 = mybir.dt.bfloat16

_CACHE = {}


def build_bass():
    nc = bacc.Bacc(None, target_bir_lowering=False)

    x_h = nc.dram_tensor("x", [C, N], F32, kind="ExternalInput")[:]
    wq_h = nc.dram_tensor("wqT", [C, C], BF16, kind="ExternalInput")[:]
    wk_h = nc.dram_tensor("wkT", [C, C], BF16, kind="ExternalInput")[:]
    wv_h = nc.dram_tensor("wvT", [C, C], BF16, kind="ExternalInput")[:]
    wo_h = nc.dram_tensor("woT", [C, C], BF16, kind="ExternalInput")[:]
    bq_h = nc.dram_tensor("bq", [C], F32, kind="ExternalInput")[:]
    bk_h = nc.dram_tensor("bk", [C], F32, kind="ExternalInput")[:]
    bv_h = nc.dram_tensor("bv", [C], F32, kind="ExternalInput")[:]
    bo_h = nc.dram_tensor("bo", [C], F32, kind="ExternalInput")[:]
    gam_h = nc.dram_tensor("gam", [C], F32, kind="ExternalInput")[:]
    bet_h = nc.dram_tensor("bet", [C], F32, kind="ExternalInput")[:]
    out_h = nc.dram_tensor("out", [C, NQ], F32, kind="ExternalOutput")[:]

    g8_np = np.zeros((P, NG), np.float32)
    g8T_np = np.zeros((NG, P), np.float32)
    for c in range(P):
        g8_np[c, c // GS] = 1.0 / GS
        g8T_np[c // GS, c] = 1.0
    g8_h = nc.inline_tensor(g8_np, name="g8")[:]
    g8T_h = nc.inline_tensor(g8T_np, name="g8T")[:]

    x_t = x_h.rearrange("(t p) n -> t p n", p=P)          # [4,128,4096]
    out_t = out_h.rearrange("(t p) n -> t p n", p=P)      # [4,128,2048]

    def col4(ap1d):
        # [512] dram vector -> [128,4] sbuf layout (column ct holds chans ct*128..)
        return bass.AP(tensor=ap1d.tensor, offset=ap1d.offset, ap=[[1, P], [P, CT]])

    with tile.TileContext(nc) as tc:
        with tc.tile_pool(name="consts", bufs=1) as cp, \
             tc.tile_pool(name="wo", bufs=1) as wop, \
             tc.tile_pool(name="qkv", bufs=1) as qkvp, \
             tc.tile_pool(name="hT", bufs=1) as hTp, \
             tc.tile_pool(name="mm", bufs=3, space="PSUM") as mmp:

            # ---- constants ----
            ones_f32 = cp.tile([P, 1], F32, tag="ones")
            nc.vector.memset(ones_f32[:], 1.0)
            ones1 = cp.tile([1, P], F32, tag="ones1")
            nc.vector.memset(ones1[:], 1.0)
            eps_t = cp.tile([P, 1], F32, tag="eps")
            nc.vector.memset(eps_t[:], EPS)
            g8_sb = cp.tile([P, NG], F32, tag="g8")
            nc.sync.dma_start(out=g8_sb[:], in_=g8_h)
            g8T_sb = cp.tile([NG, P], F32, tag="g8T")
            nc.sync.dma_start(out=g8T_sb[:], in_=g8T_h)
            bq_sb = cp.tile([P, CT], F32, tag="bq")
            nc.sync.dma_start(out=bq_sb[:], in_=col4(bq_h))
            bk_sb = cp.tile([P, CT], F32, tag="bk")
            nc.sync.dma_start(out=bk_sb[:], in_=col4(bk_h))
            bo_sb = cp.tile([P, CT], F32, tag="bo")
            nc.sync.dma_start(out=bo_sb[:], in_=col4(bo_h))
            gam_sb = cp.tile([P, CT], F32, tag="gam")
            nc.sync.dma_start(out=gam_sb[:], in_=col4(gam_h))
            bet_sb = cp.tile([P, CT], F32, tag="bet")
            nc.sync.dma_start(out=bet_sb[:], in_=col4(bet_h))
            bv_bc = cp.tile([P, C], F32, tag="bvbc")

            # ---- persistent weights / activations ----
            wo_sb = [wop.tile([P, C], BF16, tag=f"wo{t}", name=f"wo{t}") for t in range(CT)]
            wo_t = wo_h.rearrange("(t p) o -> t p o", p=P)

            q_bf = [qkvp.tile([P, NQ], BF16, tag=f"q{t}", name=f"q{t}") for t in range(CT)]
            k_bf = [qkvp.tile([P, N], BF16, tag=f"k{t}", name=f"k{t}") for t in range(CT)]
            v_bf = [qkvp.tile([P, C], BF16, tag=f"v{j}", name=f"v{j}") for j in range(JT)]
            hT_bf = [hTp.tile([P, NQ], BF16, tag=f"hT{t}", name=f"hT{t}") for t in range(CT)]

            # =========== Phase A: groupnorm -> hn (bf16), and QKV ===========
            with tc.tile_pool(name="xg", bufs=2) as xgp, \
                 tc.tile_pool(name="gnsb", bufs=2) as gnp, \
                 tc.tile_pool(name="hn", bufs=1) as hnp, \
                 tc.tile_pool(name="wqkv", bufs=1) as wqkvp, \
                 tc.tile_pool(name="gnps", bufs=2, space="PSUM") as gnps, \
                 tc.tile_pool(name="projps", bufs=3, space="PSUM") as pjp:

                wq_sb = [wqkvp.tile([P, C], BF16, tag=f"wq{t}", name=f"wq{t}") for t in range(CT)]
                wk_sb = [wqkvp.tile([P, C], BF16, tag=f"wk{t}", name=f"wk{t}") for t in range(CT)]
                wv_sb = [wqkvp.tile([P, C], BF16, tag=f"wv{t}", name=f"wv{t}") for t in range(CT)]
                wq_t = wq_h.rearrange("(t p) o -> t p o", p=P)
                wk_t = wk_h.rearrange("(t p) o -> t p o", p=P)
                wv_t = wv_h.rearrange("(t p) o -> t p o", p=P)

                hn_bf = [hnp.tile([P, N], BF16, tag=f"hn{t}", name=f"hn{t}") for t in range(CT)]

                for ct in range(CT):
                    x_sb = xgp.tile([P, N], F32, tag="x")
                    # chunked load + per-chunk stats so stats start on the
                    # first chunk instead of after the full 2MB tile
                    stats = gnp.tile([P, 8, 6], F32, tag="stats")
                    for s in range(8):
                        nc.sync.dma_start(
                            out=x_sb[:, s * 512:(s + 1) * 512],
                            in_=x_t[ct][:, s * 512:(s + 1) * 512],
                        )
                        nc.vector.bn_stats(
                            out=stats[:, s, :], in_=x_sb[:, s * 512:(s + 1) * 512]
                        )
                    mv = gnp.tile([P, 2], F32, tag="mv")
                    nc.vector.bn_aggr(out=mv[:], in_=stats[:])
                    # cstat = [mean, E[x^2]] per channel
                    cstat = gnp.tile([P, 2], F32, tag="cstat")
                    nc.vector.tensor_copy(cstat[:, 0:1], mv[:, 0:1])
                    nc.vector.tensor_mul(cstat[:, 1:2], mv[:, 0:1], mv[:, 0:1])
                    nc.vector.tensor_add(cstat[:, 1:2], cstat[:, 1:2], mv[:, 1:2])
                    # group-average then broadcast back to channels (PE)
                    psA = gnps.tile([NG, 2], F32, tag="gn")
                    nc.tensor.matmul(psA[:], lhsT=g8_sb[:], rhs=cstat[:],
                                     start=True, stop=True)
                    gt = gnp.tile([NG, 2], F32, tag="gt")
                    nc.vector.tensor_copy(gt[:], psA[:])
                    psB = gnps.tile([P, 2], F32, tag="gn")
                    nc.tensor.matmul(psB[:], lhsT=g8T_sb[:], rhs=gt[:],
                                     start=True, stop=True)
                    gstat = gnp.tile([P, 2], F32, tag="gstat")
                    nc.vector.tensor_copy(gstat[:], psB[:])
                    # a = gamma * rsqrt(gvar+eps); d = beta - gmean * a
                    vtmp = gnp.tile([P, 1], F32, tag="vtmp")
                    nc.vector.tensor_mul(vtmp[:], gstat[:, 0:1], gstat[:, 0:1])
                    nc.vector.tensor_tensor(
                        out=vtmp[:], in0=gstat[:, 1:2], in1=vtmp[:],
                        op=mybir.AluOpType.subtract,
                    )
                    nc.scalar.activation(
                        out=vtmp[:], in_=vtmp[:],
                        func=mybir.ActivationFunctionType.Sqrt,
                        bias=eps_t[:], scale=1.0,
                    )
                    rstd = gnp.tile([P, 1], F32, tag="rstd")
                    nc.vector.reciprocal(out=rstd[:], in_=vtmp[:])
                    a_t = gnp.tile([P, 1], F32, tag="a_t")
                    nc.vector.tensor_mul(a_t[:], rstd[:], gam_sb[:, ct:ct + 1])
                    d_t = gnp.tile([P, 1], F32, tag="d_t")
                    nc.vector.tensor_mul(d_t[:], gstat[:, 0:1], a_t[:])
                    nc.vector.tensor_tensor(
                        out=d_t[:], in0=bet_sb[:, ct:ct + 1], in1=d_t[:],
                        op=mybir.AluOpType.subtract,
                    )
                    for s in range(8):
                        nc.scalar.activation(
                            out=hn_bf[ct][:, s * 512:(s + 1) * 512],
                            in_=x_sb[:, s * 512:(s + 1) * 512],
                            func=mybir.ActivationFunctionType.Identity,
                            scale=a_t[:], bias=d_t[:],
                        )


                # deferred weight loads (after x so groupnorm owns DMA at t=0)
                for t in range(CT):
                    nc.sync.dma_start(out=wq_sb[t][:], in_=wq_t[t])
                    nc.sync.dma_start(out=wk_sb[t][:], in_=wk_t[t])
                    nc.sync.dma_start(out=wv_sb[t][:], in_=wv_t[t])
                    nc.sync.dma_start(out=wo_sb[t][:], in_=wo_t[t])
                nc.sync.dma_start(
                    out=bv_bc[:],
                    in_=bass.AP(tensor=bv_h.tensor, offset=bv_h.offset, ap=[[0, P], [1, C]]),
                )

                # =========== Phase B: projections ===========
                # q[ct][c, i] (2048 query tokens), k[ct][c, j] (all 4096)
                for co in range(CT):
                    for isl in range(NSTRIP):
                        ps = pjp.tile([P, 512], F32, tag="pj")
                        for t in range(CT):
                            nc.tensor.matmul(
                                ps[:],
                                lhsT=wq_sb[t][:, co * P:(co + 1) * P],
                                rhs=hn_bf[t][:, isl * 512:(isl + 1) * 512],
                                start=(t == 0), stop=(t == CT - 1),
                            )
                        nc.vector.tensor_scalar_add(
                            out=q_bf[co][:, isl * 512:(isl + 1) * 512],
                            in0=ps[:], scalar1=bq_sb[:, co:co + 1],
                        )
                    for jsl in range(N // 512):
                        ps = pjp.tile([P, 512], F32, tag="pj")
                        for t in range(CT):
                            nc.tensor.matmul(
                                ps[:],
                                lhsT=wk_sb[t][:, co * P:(co + 1) * P],
                                rhs=hn_bf[t][:, jsl * 512:(jsl + 1) * 512],
                                start=(t == 0), stop=(t == CT - 1),
                            )
                        nc.vector.tensor_scalar_add(
                            out=k_bf[co][:, jsl * 512:(jsl + 1) * 512],
                            in0=ps[:], scalar1=bk_sb[:, co:co + 1],
                        )
                # v[jt][j, c] (token-major: one matmul per 128-token tile)
                for jt in range(JT):
                    ps = mmp.tile([P, 512], F32, tag="mm")
                    for t in range(CT):
                        nc.tensor.matmul(
                            ps[:],
                            lhsT=hn_bf[t][:, jt * P:(jt + 1) * P],
                            rhs=wv_sb[t][:],
                            start=(t == 0), stop=(t == CT - 1),
                        )
                    nc.vector.tensor_tensor(
                        out=v_bf[jt][:], in0=ps[:], in1=bv_bc[:],
                        op=mybir.AluOpType.add,
                    )


            with tc.tile_pool(name="hacc", bufs=4, space="PSUM") as hp, \
                 tc.tile_pool(name="lps", bufs=1, space="PSUM") as lp, \
                 tc.tile_pool(name="attn", bufs=1) as ap_, \
                 tc.tile_pool(name="lsb", bufs=2) as lsp, \
                 tc.tile_pool(name="xres", bufs=3) as xrp, \
                 tc.tile_pool(name="outt", bufs=3) as otp:

                # =========== Phase C: attention, software-pipelined strips ===========
                pT = [ap_.tile([P, 512], BF16, tag=f"pT{j}", name=f"pT{j}") for j in range(JT)]

                def emit_strip_core(st):
                    """scores -> exp -> colsums -> l roundtrip -> h matmuls -> h evac.
                    Returns the strip's h_bf tiles (normalized, bf16)."""
                    i0 = st * 512
                    for jt in range(JT):
                        ps = mmp.tile([P, 512], F32, tag="mm", name=f"s{st}_{jt}")
                        for t in range(CT):
                            nc.tensor.matmul(
                                ps[:],
                                lhsT=k_bf[t][:, jt * P:(jt + 1) * P],
                                rhs=q_bf[t][:, i0:i0 + 512],
                                start=(t == 0), stop=(t == CT - 1),
                            )
                        nc.scalar.activation(
                            out=pT[jt][:], in_=ps[:],
                            func=mybir.ActivationFunctionType.Exp,
                            scale=SCALE,
                        )
                    acc = lsp.tile([P, 512], F32, tag="lacc", name=f"lacc{st}")
                    nc.vector.tensor_tensor(
                        out=acc[:], in0=pT[0][:], in1=pT[1][:],
                        op=mybir.AluOpType.add,
                    )
                    for jt in range(2, JT):
                        nc.vector.tensor_tensor(
                            out=acc[:], in0=acc[:], in1=pT[jt][:],
                            op=mybir.AluOpType.add,
                        )
                    psl = lp.tile([1, 512], F32, tag="l", name=f"l{st}")
                    nc.tensor.matmul(
                        psl[:], lhsT=ones_f32[:], rhs=acc[:],
                        start=True, stop=True,
                    )
                    # 1/l on the single-partition row, then broadcast to all
                    # 128 partitions with a K=1 ones-matmul (all on-chip)
                    rl1 = lsp.tile([1, 512], F32, tag="rl1")
                    nc.vector.reciprocal(out=rl1[:], in_=psl[:])
                    psb = mmp.tile([P, 512], F32, tag="mm", name=f"rlbps{st}")
                    nc.tensor.matmul(psb[:], lhsT=ones1[:], rhs=rl1[:],
                                     start=True, stop=True)
                    rlb = lsp.tile([P, 512], F32, tag="rlb", name=f"rlb{st}")
                    nc.vector.tensor_copy(rlb[:], psb[:])
                    # h^T[c, i] = sum_j v[j, c] p[j, i] -- direct hT, no transposes
                    hps = [hp.tile([P, 512], F32, tag="h", name=f"hps{st}_{i}")
                           for i in range(CT)]
                    for jt in range(JT):
                        for cb in range(CT):
                            nc.tensor.matmul(
                                hps[cb][:],
                                lhsT=v_bf[jt][:, cb * P:(cb + 1) * P],
                                rhs=pT[jt][:],
                                start=(jt == 0), stop=(jt == JT - 1),
                            )
                    # normalize + evacuate straight into hT (bf16)
                    for cb in range(CT):
                        nc.vector.tensor_mul(
                            hT_bf[cb][:, i0:i0 + 512], hps[cb][:], rlb[:]
                        )
                    return None

                def emit_strip_tail(st, h_bfs):
                    """output projection + residual for one strip."""
                    i0 = st * 512
                    for co in range(CT):
                        ps = mmp.tile([P, 512], F32, tag="mm",
                                      name=f"op{st}_{co}")
                        for t in range(CT):
                            nc.tensor.matmul(
                                ps[:],
                                lhsT=wo_sb[t][:, co * P:(co + 1) * P],
                                rhs=hT_bf[t][:, i0:i0 + 512],
                                start=(t == 0), stop=(t == CT - 1),
                            )
                        xr = xrp.tile([P, 512], F32, tag="xr")
                        nc.sync.dma_start(
                            out=xr[:], in_=x_t[co][:, i0:i0 + 512]
                        )
                        ot = otp.tile([P, 512], F32, tag="ot")
                        nc.vector.tensor_scalar_add(
                            out=ot[:], in0=ps[:], scalar1=bo_sb[:, co:co + 1]
                        )
                        nc.vector.tensor_tensor(
                            out=ot[:], in0=ot[:], in1=xr[:],
                            op=mybir.AluOpType.add,
                        )
                        nc.sync.dma_start(
                            out=out_t[co][:, i0:i0 + 512], in_=ot[:]
                        )

                prev = None
                for st in range(NSTRIP):
                    h_bfs = emit_strip_core(st)
                    if prev is not None:
                        emit_strip_tail(prev[0], prev[1])
                    prev = (st, h_bfs)
                emit_strip_tail(prev[0], prev[1])

    nc.finalize()
    return nc


def kernel(**inputs):
    if "nc" not in _CACHE:
        _CACHE["nc"] = build_bass()
    nc = _CACHE["nc"]

    x = np.ascontiguousarray(np.asarray(inputs["x"], dtype=np.float32))
    B = x.shape[0]
    xf = x.reshape(B, C, N)

    def bfT(w):
        return np.ascontiguousarray(
            np.asarray(w, dtype=np.float32).T.astype(ml_dtypes.bfloat16)
        )

    shared = {
        "wqT": bfT(inputs["wq"]), "wkT": bfT(inputs["wk"]),
        "wvT": bfT(inputs["wv"]), "woT": bfT(inputs["wo"]),
        "bq": np.ascontiguousarray(np.asarray(inputs["bq"], np.float32)),
        "bk": np.ascontiguousarray(np.asarray(inputs["bk"], np.float32)),
        "bv": np.ascontiguousarray(np.asarray(inputs["bv"], np.float32)),
        "bo": np.ascontiguousarray(np.asarray(inputs["bo"], np.float32)),
        "gam": np.ascontiguousarray(np.asarray(inputs["norm_g"], np.float32)),
        "bet": np.ascontiguousarray(np.asarray(inputs["norm_b"], np.float32)),
    }

    in_maps = []
    for core in range(2 * B):
        b, half = core // 2, core % 2
        xb = xf[b]
        if half:
            xb = np.concatenate([xb[:, NQ:], xb[:, :NQ]], axis=1)
        in_maps.append({"x": np.ascontiguousarray(xb), **shared})

    import os
    trace = bool(os.environ.get("BASS_KERNEL_TRACE"))
    res = run_bass_kernel_spmd(
        nc, in_maps, core_ids=list(range(2 * B)), trace=trace,
        trace_cores=list(range(2 * B)) if trace else None,
    )
    _CACHE["last_results"] = res

    out = np.empty((B, C, N), np.float32)
    for core in range(2 * B):
        b, half = core // 2, core % 2
        out[b][:, half * NQ:(half + 1) * NQ] = res.results[core]["out"]
    return out.reshape(B, C, 64, 64)



# revision 3
# speedup vs baseline: 1.8373x; 1.8373x over previous
"""Trainium2 Bass kernel for nn_AttnBlock (GroupNorm + single-head 4096-token
attention + residual), sharded over 8 NeuronCores.

Sharding: data-parallel over batch B=4, sequence-parallel x2 over the 4096
query tokens -> 8 shards. Each core computes k/v for its full batch
(duplicated across the 2 token-halves) and q/attention/out-proj for its 2048
query tokens. The token axis is rolled on the host for the second half so a
single SPMD NEFF serves all cores (softmax over keys is order-invariant,
groupnorm stats are token-permutation-invariant).

All six matmul groups (q/k/v projections, q@k scores, p@v, out-projection)
run in fp8 e4m3 with DoubleRow perf mode (K=256 contraction per instruction,
2x PE throughput). The softmax exp is computed with a -3 logit shift so
exp() stays under the TRN fp8e4 max of 240 (the shift cancels in the
normalization). The softmax denominator l = sum_j p is computed on the PE
with a DoubleRow ones-matmul accumulating into a [1,512] PSUM row. The v
bias is folded into the output bias on the host (bo' = bo + wo @ bv), and
the remaining biases ride the PSUM-evacuation ops (ACT activation bias /
DVE tensor_scalar).

Self-contained: hardcodes all shapes; only needs the concourse runtime.
"""

import numpy as np
import ml_dtypes

import concourse.bass as bass
import concourse.bacc as bacc
import concourse.tile as tile
from concourse import mybir
from concourse.bass_utils import run_bass_kernel_spmd

P = 128                 # partitions
C = 512                 # channels
N = 4096                # tokens (64*64)
NQ = 2048               # query tokens per core
CT = C // P             # 4 channel chunks
JT = N // P             # 32 key-token tiles of 128
UP = JT // 2            # 16 key-tile pairs (DoubleRow)
NSTRIP = NQ // 512      # 4 query strips of 512
GS = 16                 # channels per group
NG = P // GS            # 8 groups per channel chunk
EPS = 1e-6
SCALE = float(C) ** -0.5
SHIFT = -3.0            # exp(logit + SHIFT) keeps exp < 240 (fp8e4 max)
HSC = 16.0              # hT is stored as 16*h in fp8 (undone in out evac)
F32 = mybir.dt.float32
FP16 = mybir.dt.float16
FP8 = mybir.dt.float8e4
DR = mybir.MatmulPerfMode.DoubleRow

_CACHE = {}


def build_bass():
    nc = bacc.Bacc(None, target_bir_lowering=False)

    x_h = nc.dram_tensor("x", [C, N], F32, kind="ExternalInput")[:]
    wq_h = nc.dram_tensor("wqT", [C, C], FP8, kind="ExternalInput")[:]
    wk_h = nc.dram_tensor("wkT", [C, C], FP8, kind="ExternalInput")[:]
    wv_h = nc.dram_tensor("wvT", [C, C], FP8, kind="ExternalInput")[:]
    wo_h = nc.dram_tensor("woT", [C, C], FP8, kind="ExternalInput")[:]
    bq_h = nc.dram_tensor("bq", [C], F32, kind="ExternalInput")[:]
    bk_h = nc.dram_tensor("bk", [C], F32, kind="ExternalInput")[:]
    bo_h = nc.dram_tensor("bo2", [C], F32, kind="ExternalInput")[:]
    gam_h = nc.dram_tensor("gam", [C], F32, kind="ExternalInput")[:]
    bet_h = nc.dram_tensor("bet", [C], F32, kind="ExternalInput")[:]
    out_h = nc.dram_tensor("out", [C, NQ], F32, kind="ExternalOutput")[:]

    g8_np = np.zeros((P, NG), np.float32)
    g8T_np = np.zeros((NG, P), np.float32)
    for c in range(P):
        g8_np[c, c // GS] = 1.0 / GS
        g8T_np[c // GS, c] = 1.0
    g8_h = nc.inline_tensor(g8_np, name="g8")[:]
    g8T_h = nc.inline_tensor(g8T_np, name="g8T")[:]

    x_t = x_h.rearrange("(t p) n -> t p n", p=P)          # [4,128,4096]
    out_t = out_h.rearrange("(t p) n -> t p n", p=P)      # [4,128,2048]

    def col4(ap1d):
        # [512] dram vector -> [128,4] sbuf layout (column ct holds chans ct*128..)
        return bass.AP(tensor=ap1d.tensor, offset=ap1d.offset, ap=[[1, P], [P, CT]])

    with tile.TileContext(nc) as tc:
        with tc.tile_pool(name="consts", bufs=1) as cp, \
             tc.tile_pool(name="wo", bufs=1) as wop, \
             tc.tile_pool(name="qkv", bufs=1) as qkvp, \
             tc.tile_pool(name="hTp", bufs=1) as hTp, \
             tc.tile_pool(name="pTp", bufs=2) as pTp:

            # ---- constants ----
            ones2 = cp.tile([P, 2, 16], FP8, tag="ones2")
            nc.vector.memset(ones2[:], 1.0)
            ones16 = cp.tile([1, P], FP16, tag="ones16")
            nc.vector.memset(ones16[:], HSC)
            eps_t = cp.tile([P, 1], F32, tag="eps")
            nc.vector.memset(eps_t[:], EPS)
            shift_t = cp.tile([P, 1], F32, tag="shift")
            nc.vector.memset(shift_t[:], SHIFT)
            g8_sb = cp.tile([P, NG], F32, tag="g8")
            nc.sync.dma_start(out=g8_sb[:], in_=g8_h)
            g8T_sb = cp.tile([NG, P], F32, tag="g8T")
            nc.sync.dma_start(out=g8T_sb[:], in_=g8T_h)
            bq_sb = cp.tile([P, CT], F32, tag="bq")
            nc.sync.dma_start(out=bq_sb[:], in_=col4(bq_h))
            bk_sb = cp.tile([P, CT], F32, tag="bk")
            nc.sync.dma_start(out=bk_sb[:], in_=col4(bk_h))
            bo_sb = cp.tile([P, CT], F32, tag="bo")
            nc.sync.dma_start(out=bo_sb[:], in_=col4(bo_h))
            gam_sb = cp.tile([P, CT], F32, tag="gam")
            nc.sync.dma_start(out=gam_sb[:], in_=col4(gam_h))
            bet_sb = cp.tile([P, CT], F32, tag="bet")
            nc.sync.dma_start(out=bet_sb[:], in_=col4(bet_h))

            # ---- persistent fp8 activations ----
            wo_sb = wop.tile([P, CT, C], FP8, tag="wo", name="wo")
            q_sb = qkvp.tile([P, CT, NQ], FP8, tag="q", name="q")
            k_sb = qkvp.tile([P, CT, N], FP8, tag="k", name="k")
            v_sb = qkvp.tile([P, JT, C], FP8, tag="v", name="v")
            hT_sb = hTp.tile([P, CT, NQ], FP8, tag="hT", name="hT")

            wq_t = wq_h.rearrange("(t p) o -> t p o", p=P)
            wk_t = wk_h.rearrange("(t p) o -> t p o", p=P)
            wv_t = wv_h.rearrange("(t p) o -> t p o", p=P)
            wo_t = wo_h.rearrange("(t p) o -> t p o", p=P)

            # =========== Phase A: groupnorm -> hn (fp8) ===========
            with tc.tile_pool(name="xg", bufs=2) as xgp, \
                 tc.tile_pool(name="gnsb", bufs=2) as gnp, \
                 tc.tile_pool(name="hn", bufs=1) as hnp, \
                 tc.tile_pool(name="wqkv", bufs=1) as wqkvp, \
                 tc.tile_pool(name="gnps", bufs=2, space="PSUM") as gnps, \
                 tc.tile_pool(name="projps", bufs=5, space="PSUM") as pjp:

                wq_sb = wqkvp.tile([P, CT, C], FP8, tag="wq", name="wq")
                wk_sb = wqkvp.tile([P, CT, C], FP8, tag="wk", name="wk")
                wv_sb = wqkvp.tile([P, CT, C], FP8, tag="wv", name="wv")

                hn_sb = hnp.tile([P, CT, N], FP8, tag="hn", name="hn")

                for ct in range(CT):
                    x_sb = xgp.tile([P, N], F32, tag="x")
                    # chunked load + per-chunk stats so stats start on the
                    # first chunk instead of after the full 2MB tile
                    stats = gnp.tile([P, 8, 6], F32, tag="stats")
                    for s in range(8):
                        nc.sync.dma_start(
                            out=x_sb[:, s * 512:(s + 1) * 512],
                            in_=x_t[ct][:, s * 512:(s + 1) * 512],
                        )
                        nc.vector.bn_stats(
                            out=stats[:, s, :], in_=x_sb[:, s * 512:(s + 1) * 512]
                        )
                    mv = gnp.tile([P, 2], F32, tag="mv")
                    nc.vector.bn_aggr(out=mv[:], in_=stats[:])
                    # cstat = [mean, E[x^2]] per channel
                    cstat = gnp.tile([P, 2], F32, tag="cstat")
                    nc.vector.tensor_copy(cstat[:, 0:1], mv[:, 0:1])
                    nc.vector.tensor_mul(cstat[:, 1:2], mv[:, 0:1], mv[:, 0:1])
                    nc.vector.tensor_add(cstat[:, 1:2], cstat[:, 1:2], mv[:, 1:2])
                    # group-average then broadcast back to channels (PE)
                    psA = gnps.tile([NG, 2], F32, tag="gn")
                    nc.tensor.matmul(psA[:], lhsT=g8_sb[:], rhs=cstat[:],
                                     start=True, stop=True)
                    gt = gnp.tile([NG, 2], F32, tag="gt")
                    nc.vector.tensor_copy(gt[:], psA[:])
                    psB = gnps.tile([P, 2], F32, tag="gn")
                    nc.tensor.matmul(psB[:], lhsT=g8T_sb[:], rhs=gt[:],
                                     start=True, stop=True)
                    gstat = gnp.tile([P, 2], F32, tag="gstat")
                    nc.vector.tensor_copy(gstat[:], psB[:])
                    # a = gamma * rsqrt(gvar+eps); d = beta - gmean * a
                    vtmp = gnp.tile([P, 1], F32, tag="vtmp")
                    nc.vector.tensor_mul(vtmp[:], gstat[:, 0:1], gstat[:, 0:1])
                    nc.vector.tensor_tensor(
                        out=vtmp[:], in0=gstat[:, 1:2], in1=vtmp[:],
                        op=mybir.AluOpType.subtract,
                    )
                    nc.scalar.activation(
                        out=vtmp[:], in_=vtmp[:],
                        func=mybir.ActivationFunctionType.Sqrt,
                        bias=eps_t[:], scale=1.0,
                    )
                    rstd = gnp.tile([P, 1], F32, tag="rstd")
                    nc.vector.reciprocal(out=rstd[:], in_=vtmp[:])
                    a_t = gnp.tile([P, 1], F32, tag="a_t")
                    nc.vector.tensor_mul(a_t[:], rstd[:], gam_sb[:, ct:ct + 1])
                    d_t = gnp.tile([P, 1], F32, tag="d_t")
                    nc.vector.tensor_mul(d_t[:], gstat[:, 0:1], a_t[:])
                    nc.vector.tensor_tensor(
                        out=d_t[:], in0=bet_sb[:, ct:ct + 1], in1=d_t[:],
                        op=mybir.AluOpType.subtract,
                    )
                    for s in range(8):
                        nc.scalar.activation(
                            out=hn_sb[:, ct, s * 512:(s + 1) * 512],
                            in_=x_sb[:, s * 512:(s + 1) * 512],
                            func=mybir.ActivationFunctionType.Identity,
                            scale=a_t[:], bias=d_t[:],
                        )

                # deferred weight loads (after x so groupnorm owns DMA at t=0)
                for t in range(CT):
                    nc.sync.dma_start(out=wq_sb[:, t, :], in_=wq_t[t])
                    nc.sync.dma_start(out=wk_sb[:, t, :], in_=wk_t[t])
                    nc.sync.dma_start(out=wv_sb[:, t, :], in_=wv_t[t])
                    nc.sync.dma_start(out=wo_sb[:, t, :], in_=wo_t[t])

                # =========== Phase B: q/k/v projections (fp8 DR) ===========
                # Interleave the three streams so their PSUM evacuations run
                # on different engines concurrently (q,v -> ACT; k -> DVE).
                def emit_q(i):
                    co, isl = divmod(i, NSTRIP)
                    ps = pjp.tile([P, 512], F32, tag="pj", name=f"q{i}")
                    for u in range(2):
                        nc.tensor.matmul(
                            ps[:],
                            lhsT=wq_sb[:, 2 * u:2 * u + 2, co * P:(co + 1) * P],
                            rhs=hn_sb[:, 2 * u:2 * u + 2, isl * 512:(isl + 1) * 512],
                            start=(u == 0), stop=(u == 1), perf_mode=DR,
                        )
                    nc.scalar.activation(
                        out=q_sb[:, co, isl * 512:(isl + 1) * 512], in_=ps[:],
                        func=mybir.ActivationFunctionType.Identity,
                        scale=1.0, bias=bq_sb[:, co:co + 1],
                    )

                def emit_k(i):
                    co, jsl = divmod(i, 8)
                    ps = pjp.tile([P, 512], F32, tag="pj", name=f"k{i}")
                    for u in range(2):
                        nc.tensor.matmul(
                            ps[:],
                            lhsT=wk_sb[:, 2 * u:2 * u + 2, co * P:(co + 1) * P],
                            rhs=hn_sb[:, 2 * u:2 * u + 2, jsl * 512:(jsl + 1) * 512],
                            start=(u == 0), stop=(u == 1), perf_mode=DR,
                        )
                    nc.vector.tensor_scalar_add(
                        out=k_sb[:, co, jsl * 512:(jsl + 1) * 512],
                        in0=ps[:], scalar1=bk_sb[:, co:co + 1],
                    )

                def emit_v(jt):
                    # token-major v[j, c]; bias bv folded into bo' on host
                    ps = pjp.tile([P, 512], F32, tag="pj", name=f"v{jt}")
                    for u in range(2):
                        nc.tensor.matmul(
                            ps[:],
                            lhsT=hn_sb[:, 2 * u:2 * u + 2, jt * P:(jt + 1) * P],
                            rhs=wv_sb[:, 2 * u:2 * u + 2, :],
                            start=(u == 0), stop=(u == 1), perf_mode=DR,
                        )
                    nc.scalar.activation(
                        out=v_sb[:, jt, :], in_=ps[:],
                        func=mybir.ActivationFunctionType.Identity,
                        scale=1.0, bias=0.0,
                    )

                for i in range(JT):
                    emit_k(i)
                    emit_v(i)
                    if i < CT * NSTRIP:
                        emit_q(i)

            # =========== Phase C: attention, software-pipelined strips ======
            with tc.tile_pool(name="mm", bufs=3, space="PSUM") as mmp, \
                 tc.tile_pool(name="hacc", bufs=4, space="PSUM") as hp, \
                 tc.tile_pool(name="lps", bufs=1, space="PSUM") as lp, \
                 tc.tile_pool(name="lsb", bufs=2) as lsp, \
                 tc.tile_pool(name="xres", bufs=3) as xrp, \
                 tc.tile_pool(name="outt", bufs=3) as otp:

                def emit_strip_core(st):
                    i0 = st * 512
                    pT = pTp.tile([P, JT, 512], FP8, tag="pT", name=f"pT{st}")
                    # scores + exp (key-major: p^T[j, i])
                    for jt in range(JT):
                        ps = mmp.tile([P, 512], F32, tag="mm", name=f"s{st}_{jt}")
                        for u in range(2):
                            nc.tensor.matmul(
                                ps[:],
                                lhsT=k_sb[:, 2 * u:2 * u + 2, jt * P:(jt + 1) * P],
                                rhs=q_sb[:, 2 * u:2 * u + 2, i0:i0 + 512],
                                start=(u == 0), stop=(u == 1), perf_mode=DR,
                            )
                        nc.scalar.activation(
                            out=pT[:, jt, :], in_=ps[:],
                            func=mybir.ActivationFunctionType.Exp,
                            scale=SCALE, bias=shift_t[:],
                        )
                    # l = colsum(p) via DR ones-matmul; h^T via DR p@v
                    psl = lp.tile([1, 512], F32, tag="l", name=f"l{st}")
                    hps = [hp.tile([P, 512], F32, tag="h", name=f"hps{st}_{i}")
                           for i in range(CT)]
                    for u in range(UP):
                        nc.tensor.matmul(
                            psl[:], lhsT=ones2[:, :, 0:1], rhs=pT[:, 2 * u:2 * u + 2, :],
                            start=(u == 0), stop=(u == UP - 1), perf_mode=DR,
                        )
                        for cb in range(CT):
                            nc.tensor.matmul(
                                hps[cb][:],
                                lhsT=v_sb[:, 2 * u:2 * u + 2, cb * P:(cb + 1) * P],
                                rhs=pT[:, 2 * u:2 * u + 2, :],
                                start=(u == 0), stop=(u == UP - 1), perf_mode=DR,
                            )
                    # rlb = broadcast of 16/l to all partitions (fp16 matmul)
                    rl1 = lsp.tile([1, 512], FP16, tag="rl1")
                    with nc.allow_low_precision(
                        reason="1/l broadcast via fp16 matmul; fp16 mantissa "
                               "error ~5e-4 is far below the fp8 noise floor"
                    ):
                        nc.vector.reciprocal(out=rl1[:], in_=psl[:])
                    psb = mmp.tile([P, 512], F32, tag="mm", name=f"rlbps{st}")
                    nc.tensor.matmul(psb[:], lhsT=ones16[:], rhs=rl1[:],
                                     start=True, stop=True)
                    rlb = lsp.tile([P, 512], F32, tag="rlb", name=f"rlb{st}")
                    nc.vector.tensor_copy(rlb[:], psb[:])
                    # normalize + evacuate into hT (fp8, scaled by 16)
                    for cb in range(CT):
                        nc.vector.tensor_mul(
                            hT_sb[:, cb, i0:i0 + 512], hps[cb][:], rlb[:]
                        )

                def emit_strip_tail(st):
                    """output projection + residual for one strip."""
                    i0 = st * 512
                    for co in range(CT):
                        ps = mmp.tile([P, 512], F32, tag="mm",
                                      name=f"op{st}_{co}")
                        for u in range(2):
                            nc.tensor.matmul(
                                ps[:],
                                lhsT=wo_sb[:, 2 * u:2 * u + 2, co * P:(co + 1) * P],
                                rhs=hT_sb[:, 2 * u:2 * u + 2, i0:i0 + 512],
                                start=(u == 0), stop=(u == 1), perf_mode=DR,
                            )
                        xr = xrp.tile([P, 512], F32, tag="xr")
                        nc.sync.dma_start(
                            out=xr[:], in_=x_t[co][:, i0:i0 + 512]
                        )
                        ot = otp.tile([P, 512], F32, tag="ot")
                        nc.vector.tensor_scalar(
                            out=ot[:], in0=ps[:],
                            scalar1=1.0 / HSC, scalar2=bo_sb[:, co:co + 1],
                            op0=mybir.AluOpType.mult, op1=mybir.AluOpType.add,
                        )
                        nc.vector.tensor_tensor(
                            out=ot[:], in0=ot[:], in1=xr[:],
                            op=mybir.AluOpType.add,
                        )
                        nc.sync.dma_start(
                            out=out_t[co][:, i0:i0 + 512], in_=ot[:]
                        )

                prev = None
                for st in range(NSTRIP):
                    emit_strip_core(st)
                    if prev is not None:
                        emit_strip_tail(prev)
                    prev = st
                emit_strip_tail(prev)

    nc.finalize()
    return nc


def kernel(**inputs):
    if "nc" not in _CACHE:
        _CACHE["nc"] = build_bass()
    nc = _CACHE["nc"]

    x = np.ascontiguousarray(np.asarray(inputs["x"], dtype=np.float32))
    B = x.shape[0]
    xf = x.reshape(B, C, N)

    def f8T(w):
        return np.ascontiguousarray(
            np.asarray(w, dtype=np.float32).T.astype(ml_dtypes.float8_e4m3)
        )

    wo_f = np.asarray(inputs["wo"], np.float32)
    bv_f = np.asarray(inputs["bv"], np.float32)
    bo2 = np.asarray(inputs["bo"], np.float32) + wo_f @ bv_f

    shared = {
        "wqT": f8T(inputs["wq"]), "wkT": f8T(inputs["wk"]),
        "wvT": f8T(inputs["wv"]), "woT": f8T(inputs["wo"]),
        "bq": np.ascontiguousarray(np.asarray(inputs["bq"], np.float32)),
        "bk": np.ascontiguousarray(np.asarray(inputs["bk"], np.float32)),
        "bo2": np.ascontiguousarray(bo2),
        "gam": np.ascontiguousarray(np.asarray(inputs["norm_g"], np.float32)),
        "bet": np.ascontiguousarray(np.asarray(inputs["norm_b"], np.float32)),
    }

    in_maps = []
    for core in range(2 * B):
        b, half = core // 2, core % 2
        xb = xf[b]
        if half:
            xb = np.concatenate([xb[:, NQ:], xb[:, :NQ]], axis=1)
        in_maps.append({"x": np.ascontiguousarray(xb), **shared})

    import os
    trace = bool(os.environ.get("BASS_KERNEL_TRACE"))
    res = run_bass_kernel_spmd(
        nc, in_maps, core_ids=list(range(2 * B)), trace=trace,
        trace_cores=list(range(2 * B)) if trace else None,
    )
    _CACHE["last_results"] = res

    out = np.empty((B, C, N), np.float32)
    for core in range(2 * B):
        b, half = core // 2, core % 2
        out[b][:, half * NQ:(half + 1) * NQ] = res.results[core]["out"]
    return out.reshape(B, C, 64, 64)


# revision 30
# speedup vs baseline: 2.2404x; 1.2194x over previous
"""Trainium2 Bass kernel for nn_AttnBlock (GroupNorm + single-head 4096-token
attention + residual), sharded over 8 NeuronCores.

Sharding: data-parallel over batch B=4, sequence-parallel x2 over the 4096
query tokens -> 8 shards. Each core computes k/v for its full batch
(duplicated across the 2 token-halves) and q/attention/out-proj for its 2048
query tokens. The token axis is rolled on the host for the second half so a
single SPMD NEFF serves all cores (softmax over keys is order-invariant,
groupnorm stats are token-permutation-invariant).

All six matmul groups (q/k/v projections, q@k scores, p@v, out-projection)
run in fp8 e4m3 with DoubleRow perf mode (K=256 contraction per instruction,
2x PE throughput). The softmax exp runs with a -3 logit shift so exp() stays
under the TRN fp8e4 max of 240 (the shift cancels in the normalization); the
denominator l = sum_j p rides the PE as a DoubleRow ones-matmul into a
[1,512] PSUM row. The v bias is folded into the output bias on the host
(bo' = bo + wo @ bv); the other biases ride the PSUM-evacuation ops.

The attention phase is ACT-bound (the exp stream), and the PE executes its
instruction tape in order, so emission order IS the PE schedule. The tape is
laid out so nothing ever sits in front of the exp stream: projection units
are interleaved into strip 0's score octets by deadline, each strip's l/h
matmuls are staggered into the NEXT strip's score stream, and each strip's
output projection trails two strips behind. PSUM bank lifetimes: groupnorm
(2) + projections (6) during phase A/strip 0 (+ scores 2 = 8), then scores
(2) + h-acc (4) + l (1) + tail (1) from strip 1 on.

Self-contained: hardcodes all shapes; only needs the concourse runtime.
"""

import numpy as np
import ml_dtypes

import concourse.bass as bass
import concourse.bacc as bacc
import concourse.tile as tile
from concourse import mybir
from concourse.bass_utils import run_bass_kernel_spmd

P = 128                 # partitions
C = 512                 # channels
N = 4096                # tokens (64*64)
NQ = 2048               # query tokens per core
CT = C // P             # 4 channel chunks
JT = N // P             # 32 key-token tiles of 128
UP = JT // 2            # 16 key-tile pairs (DoubleRow)
NSTRIP = NQ // 512      # 4 query strips of 512
GS = 16                 # channels per group
NG = P // GS            # 8 groups per channel chunk
EPS = 1e-6
SCALE = float(C) ** -0.5
SHIFT = -3.0            # exp(logit + SHIFT) keeps exp < 240 (fp8e4 max)
HSC = 16.0              # hT is stored as 16*h in fp8 (undone in out evac)
F32 = mybir.dt.float32
FP16 = mybir.dt.float16
FP8 = mybir.dt.float8e4
DR = mybir.MatmulPerfMode.DoubleRow
IDENT = mybir.ActivationFunctionType.Identity

_CACHE = {}


def build_bass():
    nc = bacc.Bacc(None, target_bir_lowering=False)

    x_h = nc.dram_tensor("x", [C, N], F32, kind="ExternalInput")[:]
    wq_h = nc.dram_tensor("wqT", [C, C], FP8, kind="ExternalInput")[:]
    wk_h = nc.dram_tensor("wkT", [C, C], FP8, kind="ExternalInput")[:]
    wv_h = nc.dram_tensor("wvT", [C, C], FP8, kind="ExternalInput")[:]
    wo_h = nc.dram_tensor("woT", [C, C], FP8, kind="ExternalInput")[:]
    bq_h = nc.dram_tensor("bq", [C], F32, kind="ExternalInput")[:]
    bk_h = nc.dram_tensor("bk", [C], F32, kind="ExternalInput")[:]
    bo_h = nc.dram_tensor("bo2", [C], F32, kind="ExternalInput")[:]
    gam_h = nc.dram_tensor("gam", [C], F32, kind="ExternalInput")[:]
    bet_h = nc.dram_tensor("bet", [C], F32, kind="ExternalInput")[:]
    out_h = nc.dram_tensor("out", [C, NQ], F32, kind="ExternalOutput")[:]

    g8_np = np.zeros((P, NG), np.float32)
    g8T_np = np.zeros((NG, P), np.float32)
    for c in range(P):
        g8_np[c, c // GS] = 1.0 / GS
        g8T_np[c // GS, c] = 1.0
    g8_h = nc.inline_tensor(g8_np, name="g8")[:]
    g8T_h = nc.inline_tensor(g8T_np, name="g8T")[:]

    x_t = x_h.rearrange("(t p) n -> t p n", p=P)          # [4,128,4096]
    out_t = out_h.rearrange("(t p) n -> t p n", p=P)      # [4,128,2048]

    def col4(ap1d):
        # [512] dram vector -> [128,4] sbuf layout (column ct holds chans ct*128..)
        return bass.AP(tensor=ap1d.tensor, offset=ap1d.offset, ap=[[1, P], [P, CT]])

    with tile.TileContext(nc) as tc:
        with tc.tile_pool(name="consts", bufs=1) as cp, \
             tc.tile_pool(name="xp", bufs=1) as xp, \
             tc.tile_pool(name="wo", bufs=1) as wop, \
             tc.tile_pool(name="wqkv", bufs=1) as wqkvp, \
             tc.tile_pool(name="hn", bufs=1) as hnp, \
             tc.tile_pool(name="qkv", bufs=1) as qkvp, \
             tc.tile_pool(name="hTp", bufs=1) as hTp, \
             tc.tile_pool(name="pTp", bufs=2) as pTp, \
             tc.tile_pool(name="lsb", bufs=2) as lsp, \
             tc.tile_pool(name="outt", bufs=8) as otp, \
             tc.tile_pool(name="mm", bufs=3, space="PSUM") as mmp:

            # ---- constants ----
            ones2 = cp.tile([P, 2, 16], FP8, tag="ones2")
            nc.vector.memset(ones2[:], 1.0)
            ones16 = cp.tile([1, P], FP16, tag="ones16")
            nc.vector.memset(ones16[:], HSC)
            eps_t = cp.tile([P, 1], F32, tag="eps")
            nc.vector.memset(eps_t[:], EPS)
            shift_t = cp.tile([P, 1], F32, tag="shift")
            nc.vector.memset(shift_t[:], SHIFT)
            g8_sb = cp.tile([P, NG], F32, tag="g8")
            nc.sync.dma_start(out=g8_sb[:], in_=g8_h)
            g8T_sb = cp.tile([NG, P], F32, tag="g8T")
            nc.sync.dma_start(out=g8T_sb[:], in_=g8T_h)
            bq_sb = cp.tile([P, CT], F32, tag="bq")
            nc.sync.dma_start(out=bq_sb[:], in_=col4(bq_h))
            bk_sb = cp.tile([P, CT], F32, tag="bk")
            nc.sync.dma_start(out=bk_sb[:], in_=col4(bk_h))
            bo_sb = cp.tile([P, CT], F32, tag="bo")
            nc.sync.dma_start(out=bo_sb[:], in_=col4(bo_h))
            gam_sb = cp.tile([P, CT], F32, tag="gam")
            nc.sync.dma_start(out=gam_sb[:], in_=col4(gam_h))
            bet_sb = cp.tile([P, CT], F32, tag="bet")
            nc.sync.dma_start(out=bet_sb[:], in_=col4(bet_h))

            # ---- persistent tiles ----
            # x stays resident: residual reads it from SBUF (no reload DMA)
            x_sb = [xp.tile([P, N], F32, tag=f"x{t}", name=f"x{t}")
                    for t in range(CT)]
            wo_sb = wop.tile([P, CT, C], FP8, tag="wo", name="wo")
            wq_sb = wqkvp.tile([P, CT, C], FP8, tag="wq", name="wq")
            wk_sb = wqkvp.tile([P, CT, C], FP8, tag="wk", name="wk")
            wv_sb = wqkvp.tile([P, CT, C], FP8, tag="wv", name="wv")
            hn_sb = hnp.tile([P, CT, N], FP8, tag="hn", name="hn")
            q_sb = qkvp.tile([P, CT, NQ], FP8, tag="q", name="q")
            k_sb = qkvp.tile([P, CT, N], FP8, tag="k", name="k")
            v_sb = qkvp.tile([P, JT, C], FP8, tag="v", name="v")
            hT_sb = hTp.tile([P, CT, NQ], FP8, tag="hT", name="hT")

            wq_t = wq_h.rearrange("(t p) o -> t p o", p=P)
            wk_t = wk_h.rearrange("(t p) o -> t p o", p=P)
            wv_t = wv_h.rearrange("(t p) o -> t p o", p=P)
            wo_t = wo_h.rearrange("(t p) o -> t p o", p=P)

            # weights issue on the Activation hwdge queue so the SP queue
            # streams x from the first cycle
            for t in range(CT):
                nc.scalar.dma_start(out=wq_sb[:, t, :], in_=wq_t[t])
                nc.scalar.dma_start(out=wk_sb[:, t, :], in_=wk_t[t])
                nc.scalar.dma_start(out=wv_sb[:, t, :], in_=wv_t[t])
                nc.scalar.dma_start(out=wo_sb[:, t, :], in_=wo_t[t])

            # =========== Phase A: groupnorm -> hn (fp8) ===========
            with tc.tile_pool(name="gnsb", bufs=2) as gnp, \
                 tc.tile_pool(name="gnps", bufs=2, space="PSUM") as gnps:

                for ct in range(CT):
                    # 2048-col DMA chunks; bn_stats stays at 512 (HW limit)
                    stats = gnp.tile([P, 8, 6], F32, tag="stats")
                    for half in range(2):
                        nc.sync.dma_start(
                            out=x_sb[ct][:, half * NQ:(half + 1) * NQ],
                            in_=x_t[ct][:, half * NQ:(half + 1) * NQ],
                        )
                        for s in range(4 * half, 4 * half + 4):
                            nc.vector.bn_stats(
                                out=stats[:, s, :],
                                in_=x_sb[ct][:, s * 512:(s + 1) * 512],
                            )
                    mv = gnp.tile([P, 2], F32, tag="mv")
                    nc.vector.bn_aggr(out=mv[:], in_=stats[:])
                    # cstat = [mean, E[x^2]] per channel
                    cstat = gnp.tile([P, 2], F32, tag="cstat")
                    nc.vector.tensor_copy(cstat[:, 0:1], mv[:, 0:1])
                    nc.vector.tensor_mul(cstat[:, 1:2], mv[:, 0:1], mv[:, 0:1])
                    nc.vector.tensor_add(cstat[:, 1:2], cstat[:, 1:2], mv[:, 1:2])
                    # group-average then broadcast back to channels (PE)
                    psA = gnps.tile([NG, 2], F32, tag="gn")
                    nc.tensor.matmul(psA[:], lhsT=g8_sb[:], rhs=cstat[:],
                                     start=True, stop=True)
                    gt = gnp.tile([NG, 2], F32, tag="gt")
                    nc.vector.tensor_copy(gt[:], psA[:])
                    psB = gnps.tile([P, 2], F32, tag="gn")
                    nc.tensor.matmul(psB[:], lhsT=g8T_sb[:], rhs=gt[:],
                                     start=True, stop=True)
                    gstat = gnp.tile([P, 2], F32, tag="gstat")
                    nc.vector.tensor_copy(gstat[:], psB[:])
                    # a = gamma * rsqrt(gvar+eps); d = beta - gmean * a
                    vtmp = gnp.tile([P, 1], F32, tag="vtmp")
                    nc.vector.tensor_mul(vtmp[:], gstat[:, 0:1], gstat[:, 0:1])
                    nc.vector.tensor_tensor(
                        out=vtmp[:], in0=gstat[:, 1:2], in1=vtmp[:],
                        op=mybir.AluOpType.subtract,
                    )
                    nc.scalar.activation(
                        out=vtmp[:], in_=vtmp[:],
                        func=mybir.ActivationFunctionType.Sqrt,
                        bias=eps_t[:], scale=1.0,
                    )
                    rstd = gnp.tile([P, 1], F32, tag="rstd")
                    nc.vector.reciprocal(out=rstd[:], in_=vtmp[:])
                    a_t = gnp.tile([P, 1], F32, tag="a_t")
                    nc.vector.tensor_mul(a_t[:], rstd[:], gam_sb[:, ct:ct + 1])
                    d_t = gnp.tile([P, 1], F32, tag="d_t")
                    nc.vector.tensor_mul(d_t[:], gstat[:, 0:1], a_t[:])
                    nc.vector.tensor_tensor(
                        out=d_t[:], in0=bet_sb[:, ct:ct + 1], in1=d_t[:],
                        op=mybir.AluOpType.subtract,
                    )
                    nc.scalar.activation(
                        out=hn_sb[:, ct, :], in_=x_sb[ct][:, :],
                        func=IDENT, scale=a_t[:], bias=d_t[:],
                    )

            # =========== Phase B unit definitions (fp8 DR, paired PSUM) ====
            toggle = [0]

            def nxt():
                toggle[0] += 1
                return "dve" if toggle[0] % 2 else "pool"

            def evac(ps_ap, out_ap, bias, eng):
                # GPSIMD cannot read PSUM on HW: evacs are DVE or ACT only
                if eng == "act":
                    nc.scalar.activation(out=out_ap, in_=ps_ap, func=IDENT,
                                         scale=1.0,
                                         bias=0.0 if bias is None else bias)
                elif bias is None:
                    nc.vector.tensor_copy(out_ap, ps_ap)
                else:
                    nc.vector.tensor_scalar_add(out=out_ap, in0=ps_ap,
                                                scalar1=bias)

            def make_units(pjp):
                def unit_v(t2, eng):
                    # token-major v[j, c] for key tiles 2*t2, 2*t2+1
                    ps = pjp.tile([P, 1024], F32, tag="pj", name=f"vps{t2}")
                    for h_ in range(2):
                        jt = 2 * t2 + h_
                        for u in range(2):
                            nc.tensor.matmul(
                                ps[:, h_ * 512:(h_ + 1) * 512],
                                lhsT=hn_sb[:, 2 * u:2 * u + 2,
                                           jt * P:(jt + 1) * P],
                                rhs=wv_sb[:, 2 * u:2 * u + 2, :],
                                start=(u == 0), stop=(u == 1), perf_mode=DR,
                            )
                    # bias bv folded into bo' on host
                    evac(ps[:, 0:1024], v_sb[:, 2 * t2:2 * t2 + 2, :],
                         None, eng)

                def unit_k(co, jp_, eng):
                    ps = pjp.tile([P, 1024], F32, tag="pj",
                                  name=f"kps{co}_{jp_}")
                    for h_ in range(2):
                        jsl = 2 * jp_ + h_
                        for u in range(2):
                            nc.tensor.matmul(
                                ps[:, h_ * 512:(h_ + 1) * 512],
                                lhsT=wk_sb[:, 2 * u:2 * u + 2,
                                           co * P:(co + 1) * P],
                                rhs=hn_sb[:, 2 * u:2 * u + 2,
                                          jsl * 512:(jsl + 1) * 512],
                                start=(u == 0), stop=(u == 1), perf_mode=DR,
                            )
                    evac(ps[:, 0:1024],
                         k_sb[:, co, jp_ * 1024:(jp_ + 1) * 1024],
                         bk_sb[:, co:co + 1], eng)

                def unit_q(co, ip_, eng):
                    ps = pjp.tile([P, 1024], F32, tag="pj",
                                  name=f"qps{co}_{ip_}")
                    for h_ in range(2):
                        isl = 2 * ip_ + h_
                        for u in range(2):
                            nc.tensor.matmul(
                                ps[:, h_ * 512:(h_ + 1) * 512],
                                lhsT=wq_sb[:, 2 * u:2 * u + 2,
                                           co * P:(co + 1) * P],
                                rhs=hn_sb[:, 2 * u:2 * u + 2,
                                          isl * 512:(isl + 1) * 512],
                                start=(u == 0), stop=(u == 1), perf_mode=DR,
                            )
                    evac(ps[:, 0:1024],
                         q_sb[:, co, ip_ * 1024:(ip_ + 1) * 1024],
                         bq_sb[:, co:co + 1], eng)

                return unit_v, unit_k, unit_q

            # late q units (strips 2-3) run through the tail PSUM bank during
            # strip 1, after the projection pool has closed
            def unit_q_late(co, isl, pool):
                ps = pool.tile([P, 512], F32, tag="mm", name=f"qL{co}_{isl}")
                for u in range(2):
                    nc.tensor.matmul(
                        ps[:],
                        lhsT=wq_sb[:, 2 * u:2 * u + 2, co * P:(co + 1) * P],
                        rhs=hn_sb[:, 2 * u:2 * u + 2, isl * 512:(isl + 1) * 512],
                        start=(u == 0), stop=(u == 1), perf_mode=DR,
                    )
                evac(ps[:, 0:512], q_sb[:, co, isl * 512:(isl + 1) * 512],
                     bq_sb[:, co:co + 1], "dve")

            # =========== Phase C: strip tape ================================
            def emit_score_octet(st, pT, g):
                i0 = st * 512
                for jt in range(8 * g, 8 * g + 8):
                    ps = mmp.tile([P, 512], F32, tag="mm", name=f"s{st}_{jt}")
                    for u in range(2):
                        nc.tensor.matmul(
                            ps[:],
                            lhsT=k_sb[:, 2 * u:2 * u + 2, jt * P:(jt + 1) * P],
                            rhs=q_sb[:, 2 * u:2 * u + 2, i0:i0 + 512],
                            start=(u == 0), stop=(u == 1), perf_mode=DR,
                        )
                    nc.scalar.activation(
                        out=pT[:, jt, :], in_=ps[:],
                        func=mybir.ActivationFunctionType.Exp,
                        scale=SCALE, bias=shift_t[:],
                    )

            def emit_lh_quarter(st, pT, psl, hps, qq):
                for u in range(4 * qq, 4 * qq + 4):
                    nc.tensor.matmul(
                        psl[:], lhsT=ones2[:, :, 0:1],
                        rhs=pT[:, 2 * u:2 * u + 2, :],
                        start=(u == 0), stop=(u == UP - 1), perf_mode=DR,
                    )
                    for cb in range(CT):
                        nc.tensor.matmul(
                            hps[cb][:],
                            lhsT=v_sb[:, 2 * u:2 * u + 2, cb * P:(cb + 1) * P],
                            rhs=pT[:, 2 * u:2 * u + 2, :],
                            start=(u == 0), stop=(u == UP - 1), perf_mode=DR,
                        )

            def emit_lchain(st, psl, hps):
                i0 = st * 512
                rl1 = lsp.tile([1, 512], FP16, tag="rl1")
                with nc.allow_low_precision(
                    reason="1/l via fp16; mantissa error ~5e-4 is far below "
                           "the fp8 noise floor"
                ):
                    nc.vector.reciprocal(out=rl1[:], in_=psl[:])
                psb = mmp.tile([P, 512], F32, tag="mm", name=f"rlbps{st}")
                nc.tensor.matmul(psb[:], lhsT=ones16[:], rhs=rl1[:],
                                 start=True, stop=True)
                rlb = lsp.tile([P, 512], F32, tag="rlb", name=f"rlb{st}")
                nc.vector.tensor_copy(rlb[:], psb[:])
                for cb in range(CT):
                    nc.vector.tensor_mul(
                        hT_sb[:, cb, i0:i0 + 512], hps[cb][:], rlb[:]
                    )

            def emit_tail(st, pool, last=False, tag="mm"):
                """output projection + residual for one strip."""
                i0 = st * 512
                for co in range(CT):
                    ps = pool.tile([P, 512], F32, tag=tag,
                                   name=f"op{st}_{co}")
                    for u in range(2):
                        nc.tensor.matmul(
                            ps[:],
                            lhsT=wo_sb[:, 2 * u:2 * u + 2, co * P:(co + 1) * P],
                            rhs=hT_sb[:, 2 * u:2 * u + 2, i0:i0 + 512],
                            start=(u == 0), stop=(u == 1), perf_mode=DR,
                        )
                    ot = otp.tile([P, 512], F32, tag="ot")
                    if last and co % 2 == 0:
                        # ACT is idle during the drain; split the chain
                        nc.scalar.activation(
                            out=ot[:], in_=ps[:], func=IDENT,
                            scale=1.0 / HSC, bias=bo_sb[:, co:co + 1],
                        )
                    else:
                        nc.vector.tensor_scalar(
                            out=ot[:], in0=ps[:],
                            scalar1=1.0 / HSC, scalar2=bo_sb[:, co:co + 1],
                            op0=mybir.AluOpType.mult, op1=mybir.AluOpType.add,
                        )
                    radd = nc.gpsimd if co % 2 else nc.vector
                    radd.tensor_tensor(
                        out=ot[:], in0=ot[:], in1=x_sb[co][:, i0:i0 + 512],
                        op=mybir.AluOpType.add,
                    )
                    nc.sync.dma_start(
                        out=out_t[co][:, i0:i0 + 512], in_=ot[:]
                    )

            pTs = [None] * NSTRIP

            # ---- strip 0: scores + projection units interleaved ----
            with tc.tile_pool(name="projps", bufs=2, space="PSUM") as pjp:
                unit_v, unit_k, unit_q = make_units(pjp)
                # pre-strip: the units that gate the first exps.
                # k/q evacs stay on DVE (faster per op) so the exp stream is
                # never gated by the slower Pool engine; v rides Pool.
                for co in range(CT):
                    unit_q(co, 0, "act")
                for co in range(CT):
                    unit_k(co, 0, "dve")
                for t2 in range(3):
                    unit_v(t2, "dve")

                pTs[0] = pTp.tile([P, JT, 512], FP8, tag="pT", name="pT0")
                for g in range(4):
                    emit_score_octet(0, pTs[0], g)
                    if g < 3:                      # k jp1..3 by deadline
                        for co in range(CT):
                            unit_k(co, g + 1, "dve")
                        # one of each v pair rides the ACT stream: DVE is the
                        # congested engine during strip 0
                        unit_v(3 + 2 * g, "act")
                        unit_v(4 + 2 * g, "dve")
                for t2 in range(9, UP):            # v tail
                    unit_v(t2, "dve")

            # ---- strips 1..3: staggered l/h + tails ----
            with tc.tile_pool(name="hacc", bufs=4, space="PSUM") as hp, \
                 tc.tile_pool(name="lps", bufs=1, space="PSUM") as lp:
                tpp = mmp

                def open_lh(st):
                    psl = lp.tile([1, 512], F32, tag="l", name=f"l{st}")
                    hps = [hp.tile([P, 512], F32, tag="h", name=f"hps{st}_{i}")
                           for i in range(CT)]
                    return psl, hps

                for st in range(1, NSTRIP):
                    pTs[st] = pTp.tile([P, JT, 512], FP8, tag="pT",
                                       name=f"pT{st}")
                    psl, hps = open_lh(st - 1)
                    for g in range(4):
                        emit_score_octet(st, pTs[st], g)
                        if st == 1:                # late q (strips 2-3)
                            isl = 2 + g // 2
                            for co in (0, 1) if g % 2 == 0 else (2, 3):
                                unit_q_late(co, isl, tpp)
                            emit_lh_quarter(st - 1, pTs[st - 1], psl, hps, g)
                        else:
                            if g == 0:             # tail trails two strips
                                emit_tail(st - 2, tpp)
                                for qq in range(4):
                                    emit_lh_quarter(st - 1, pTs[st - 1],
                                                    psl, hps, qq)
                                emit_lchain(st - 1, psl, hps)
                            elif g == 2 and st == NSTRIP - 1:
                                emit_tail(st - 1, tpp)
                    if st == 1:
                        emit_lchain(st - 1, psl, hps)

                # post: last strip's l/h + remaining tails
                psl, hps = open_lh(NSTRIP - 1)
                for g in range(4):
                    emit_lh_quarter(NSTRIP - 1, pTs[NSTRIP - 1], psl, hps, g)
                emit_lchain(NSTRIP - 1, psl, hps)
                emit_tail(NSTRIP - 1, hp, last=True, tag="h")
                del tpp

    nc.finalize()
    return nc


def kernel(**inputs):
    if "nc" not in _CACHE:
        _CACHE["nc"] = build_bass()
    nc = _CACHE["nc"]

    x = np.ascontiguousarray(np.asarray(inputs["x"], dtype=np.float32))
    B = x.shape[0]
    xf = x.reshape(B, C, N)

    def f8T(w):
        return np.ascontiguousarray(
            np.asarray(w, dtype=np.float32).T.astype(ml_dtypes.float8_e4m3)
        )

    wo_f = np.asarray(inputs["wo"], np.float32)
    bv_f = np.asarray(inputs["bv"], np.float32)
    bo2 = np.asarray(inputs["bo"], np.float32) + wo_f @ bv_f

    shared = {
        "wqT": f8T(inputs["wq"]), "wkT": f8T(inputs["wk"]),
        "wvT": f8T(inputs["wv"]), "woT": f8T(inputs["wo"]),
        "bq": np.ascontiguousarray(np.asarray(inputs["bq"], np.float32)),
        "bk": np.ascontiguousarray(np.asarray(inputs["bk"], np.float32)),
        "bo2": np.ascontiguousarray(bo2),
        "gam": np.ascontiguousarray(np.asarray(inputs["norm_g"], np.float32)),
        "bet": np.ascontiguousarray(np.asarray(inputs["norm_b"], np.float32)),
    }

    in_maps = []
    for core in range(2 * B):
        b, half = core // 2, core % 2
        xb = xf[b]
        if half:
            xb = np.concatenate([xb[:, NQ:], xb[:, :NQ]], axis=1)
        in_maps.append({"x": np.ascontiguousarray(xb), **shared})

    import os
    trace = bool(os.environ.get("BASS_KERNEL_TRACE"))
    res = run_bass_kernel_spmd(
        nc, in_maps, core_ids=list(range(2 * B)), trace=trace,
        trace_cores=list(range(2 * B)) if trace else None,
    )
    _CACHE["last_results"] = res

    out = np.empty((B, C, N), np.float32)
    for core in range(2 * B):
        b, half = core // 2, core % 2
        out[b][:, half * NQ:(half + 1) * NQ] = res.results[core]["out"]
    return out.reshape(B, C, 64, 64)
